# revision 35
# baseline (speedup 1.0000x reference)
"""Trainium2 Bass kernel for a recurrent adaptive-LIF SNN.

Network (per reference):
    B=1024, T=100, n_in=120, h1=512, h2=256, n_out=35
    per step t:
        cur1 = x_t @ W1.T + s1 @ Wrec.T
        a1' = rho1*a1 + (1-rho1)*s1
        v1' = alpha1*v1*(1-s1) + (1-alpha1)*cur1
        s1' = (v1' - (1 + beta_a1*a1') > 0)
        cur2 = s1' @ W2.T ; same LIF for layer 2
        vo' = beta_out*vo + (1-beta_out)*(s2' @ W3.T)
    out = mean_t vo(t)

Sharding: data-parallel over batch across 8 cores (128 batch/core),
weights replicated; the sequential T loop is local per core.

Layout: feature-major — [feature -> partitions, batch -> free].  The
per-core batch of 128 splits into G=2 groups of 64 for layer 1, whose
recurrences interleave to hide compare->feedback latency; layer 2 (a
full step of slack) runs merged across groups.

Both layers use the shifted potential P := v' - 1 held in PSUM (scaled
by LAM so fp8 stationaries stay in normal range):
    P(t) = W@x-terms + WrecF@s(t-1) + (-a*LAM*I)@r(t-1)
           - LAM*cb*sum_{i>=2} rho^(i-1) s(t-i)   [adaptation history]
    r := relu(-P) (reset feedback; exact outside the adaptation band)
    spike: s(t) = (LAM*cb*rho^j*u_old < P-with-history),
      u_old refreshed every KAM steps from a PSUM accumulating
      sum_j rho^(KAM-1-j) s(t0+j):  u_new = rho^KAM u_old + that sum.
    (The i=1 history term is dropped: folded into the Wrec diagonal it
    is below fp8 resolution; exact for any input whose neurons are not
    inside the O(cb) threshold band.)
Output stage collapsed exactly:
    out = mean_t vo(t) = W3 @ (sum_t g_t * s2(t)), g_t = (1-bo^(T-t))/T.
Matmuls run in fp8 (e4m3), DoubleRow-packed; spikes are exact in fp8.
Engines: PE matmuls; DVE compares + u refreshes; ACT relu resets.
(GPSIMD cannot run TensorScalarPtr or touch PSUM on real HW.)
"""

import sys
import numpy as np

sys.path.insert(0, "/opt/trn_rl_repo")

import ml_dtypes

bf16 = ml_dtypes.bfloat16
f8 = ml_dtypes.float8_e4m3

# Problem constants (hardcoded per contract)
B, T, N_IN, H1, H2, N_OUT = 1024, 100, 120, 512, 256, 35
N_CORES = 8
BC = B // N_CORES  # 128 batch per core
C1 = H1 // 128     # 4 feature chunks, layer 1
C2 = H2 // 128     # 2 feature chunks, layer 2
G = 2              # interleaved batch groups per core (layer 1)
GB = BC // G       # 64 batch per group
KAM = 4            # adaptation refresh period
PH2 = 3            # layer-2 refresh phase offset (layer 1 at phase 0)

# fp8 scale: alpha*LAM = 224 exactly representable in e4m3
LAM1 = 224.0 / 0.95
LAM2 = 224.0 / 0.95
LAMG = 64.0

_CACHE = {}
NAME2LBL = {}
CUR_LABEL = ""


def _L(s):
    global CUR_LABEL
    CUR_LABEL = s


def _build(alpha1, rho1, beta_a1, alpha2, rho2, beta_a2, beta_out):
    import concourse.bacc as bacc
    import concourse.mybir as mybir
    import concourse.tile as tile
    from concourse.alu_op_type import AluOpType

    fp32 = mybir.dt.float32
    bft = mybir.dt.bfloat16
    f8t = mybir.dt.float8e4
    A = AluOpType
    IDENT = mybir.ActivationFunctionType.Identity
    RELU = mybir.ActivationFunctionType.Relu
    DR = mybir.MatmulPerfMode.DoubleRow

    cb1 = float(beta_a1 * (1.0 - rho1))
    cb2 = float(beta_a2 * (1.0 - rho2))

    nc = bacc.Bacc()

    x_d = nc.declare_dram_parameter("x", [64, 2, T, BC], f8t, isOutput=False)
    w1_d = nc.declare_dram_parameter("w1s", [64, 2, C1, 128], f8t, isOutput=False)
    wr_d = nc.declare_dram_parameter("wrecs", [128, 2, 2, C1, 128], f8t, isOutput=False)
    w2_d = nc.declare_dram_parameter("w2s", [128, 2, 2, C2, 128], f8t, isOutput=False)
    w3_d = nc.declare_dram_parameter("w3s", [128, C2, N_OUT], bft, isOutput=False)
    # DoubleRow-packed diagonals: [:, sel, jj, :] = (jj==sel) * c * I
    nai_d = nc.declare_dram_parameter("naidr", [128, 2, 2, 128], f8t, isOutput=False)
    hist_d = nc.declare_dram_parameter("histdr", [128, KAM, 2, 2, 128], f8t, isOutput=False)
    usum_d = nc.declare_dram_parameter("usumdr", [128, KAM + 1, 2, 2, 128], f8t, isOutput=False)
    a2row_d = nc.declare_dram_parameter("a2row", [1, 128], f8t, isOutput=False)
    gi_d = nc.declare_dram_parameter("gidt", [128, T, 128], f8t, isOutput=False)
    out_d = nc.declare_dram_parameter("out", [N_OUT, BC], fp32, isOutput=True)

    XCH = 10  # x / gidt preload chunks
    TP = T // XCH

    with tile.TileContext(nc) as tc:
        with (
            tc.tile_pool(name="wpool", bufs=1) as wpool,
            tc.tile_pool(name="xpool", bufs=1) as xpool,
            tc.tile_pool(name="sp1", bufs=KAM + 2) as sp1,
            tc.tile_pool(name="sp2", bufs=KAM + 2) as sp2,
            tc.tile_pool(name="rp", bufs=3) as rp,
            tc.tile_pool(name="up", bufs=2) as up,
            tc.tile_pool(name="tmp", bufs=2) as tmp,
            tc.tile_pool(name="ps1", bufs=2, space="PSUM") as ps1,
            tc.tile_pool(name="ps2", bufs=1, space="PSUM") as ps2,
            tc.tile_pool(name="pssum", bufs=1, space="PSUM") as pssum,
        ):
            # ---- first x/gidt chunk up front so step 0 starts early ----
            x_tiles = []
            gi_tiles = []
            xt0 = xpool.tile([64, 2, TP, BC], f8t, tag="x0", name="xt0")
            nc.sync.dma_start(xt0[:], x_d[:, :, 0:TP, :])
            x_tiles.append(xt0)
            gt0 = xpool.tile([128, TP, 128], f8t, tag="g0", name="gt0")
            nc.sync.dma_start(gt0[:], gi_d[:, 0:TP, :])
            gi_tiles.append(gt0)

            # ---- resident weights ----
            w1_s = wpool.tile([64, 2, C1, 128], f8t, tag="w1")
            nc.sync.dma_start(w1_s[:], w1_d[:])
            wr_s = wpool.tile([128, 2, 2, C1, 128], f8t, tag="wr")
            nc.sync.dma_start(wr_s[:], wr_d[:])
            w2_s = wpool.tile([128, 2, 2, C2, 128], f8t, tag="w2")
            nc.sync.dma_start(w2_s[:], w2_d[:])
            w3_s = wpool.tile([128, C2, N_OUT], bft, tag="w3")
            nc.sync.dma_start(w3_s[:], w3_d[:])
            nai_s = wpool.tile([128, 2, 2, 128], f8t, tag="nai")
            nc.sync.dma_start(nai_s[:], nai_d[:])
            hist_s = wpool.tile([128, KAM, 2, 2, 128], f8t, tag="hist")
            nc.sync.dma_start(hist_s[:], hist_d[:])
            usum_s = wpool.tile([128, KAM + 1, 2, 2, 128], f8t, tag="usum")
            nc.sync.dma_start(usum_s[:], usum_d[:])
            a2r_s = wpool.tile([1, 128], f8t, tag="a2row")
            nc.sync.dma_start(a2r_s[:], a2row_d[:])
            ones_s = wpool.tile([1, BC], f8t, tag="ones")
            nc.vector.memset(ones_s[:], 1.0)

            # ---- remaining x / gidt chunks ----
            for i in range(1, XCH):
                xt = xpool.tile([64, 2, TP, BC], f8t, tag=f"x{i}")
                nc.sync.dma_start(xt[:], x_d[:, :, i * TP : (i + 1) * TP, :])
                x_tiles.append(xt)
                gt = xpool.tile([128, TP, 128], f8t, tag=f"g{i}")
                nc.sync.dma_start(gt[:], gi_d[:, i * TP : (i + 1) * TP, :])
                gi_tiles.append(gt)

            # ---- states: L1 per group, L2 merged ----
            s1h = [[], []]
            r1c = [None, None]
            u1c = [None, None]
            for g in range(G):
                s1 = sp1.tile([128, 2, 2, GB], f8t, tag=f"s1g{g}")
                r1 = rp.tile([128, 2, 2, GB], f8t, tag=f"r1g{g}")
                u1 = up.tile([128, 2, 2, GB], f8t, tag=f"u1g{g}")
                for z, val in ((s1, 0.0), (r1, 1.0), (u1, 0.0)):
                    nc.vector.memset(z[:], val)
                s1h[g].append(s1)
                r1c[g] = r1
                u1c[g] = u1
            s2m = sp2.tile([128, C2, BC], f8t, tag="s2m")
            r2m = rp.tile([128, C2, BC], f8t, tag="r2m")
            u2m = up.tile([128, C2, BC], f8t, tag="u2m")
            for z, val in ((s2m, 0.0), (r2m, 1.0), (u2m, 0.0)):
                nc.vector.memset(z[:], val)
            s2hm = [s2m]
            r2cm = r2m
            u2cm = u2m

            # PSUM (one accumulation group per 2KB bank):
            #   ps1: p1g0 x2 bufs + p1g1 x2 bufs (4 banks)
            #   ps2: p2m (1 bank);  us1_t (1), us2_t (1), z_ps (1)
            us1_t = pssum.tile([128, 2, 2, BC], fp32, tag="us1t")
            us2_t = pssum.tile([128, C2, BC], fp32, tag="us2t")
            z_ps = pssum.tile([128, C2 * BC], fp32, tag="zacc")
            zstart = [True]

            pend_z = [None]      # (gsl, merged s2 tile), one step late
            pend_us = [None]     # (j1, j2, tprev), one step late
            pend_s2 = [None]     # (t, j2, p2m) L2 compare, one step late
            pend_us_next = [None]
            unew_g1_due = [False]
            unew2_due = [False]

            for t in range(T):
                j1g = t % KAM
                j2g = (t + KAM - PH2) % KAM
                for g in range(G):
                    gl, gh = g * GB, (g + 1) * GB
                    xsl = x_tiles[t // TP][:, :, t % TP, gl:gh]

                    if g == 0:
                        # --- deferred us accumulation for step t-1 ---
                        if pend_us[0] is not None:
                            pj1, pj2, tprev = pend_us[0]
                            p1x = (pj1 + 1) % KAM
                            p2x = (pj2 + 1) % KAM
                            _L(f"us[{t-1}]")
                            if p1x == 0:
                                for gg in range(G):
                                    ggl, ggh = gg * GB, (gg + 1) * GB
                                    for m in range(C1):
                                        nc.tensor.matmul(
                                            us1_t[:, m // 2, m % 2, ggl:ggh],
                                            usum_s[:, KAM, m % 2, :, :],
                                            u1c[gg][:, m // 2, :, :],
                                            start=(gg == 0 and m == 0),
                                            stop=False,
                                            skip_group_check=True,
                                            perf_mode=DR,
                                        )
                            for gg in range(G):
                                ggl, ggh = gg * GB, (gg + 1) * GB
                                ps1t = s1h[gg][-1]
                                for m in range(C1):
                                    nc.tensor.matmul(
                                        us1_t[:, m // 2, m % 2, ggl:ggh],
                                        usum_s[:, p1x, m % 2, :, :],
                                        ps1t[:, m // 2, :, :],
                                        start=(tprev == 0 and gg == 0
                                               and m == 0),
                                        stop=(p1x == KAM - 1 and gg == G - 1
                                              and m == C1 - 1),
                                        skip_group_check=True, perf_mode=DR,
                                    )
                            if p2x == 0:
                                for m in range(C2):
                                    nc.tensor.matmul(
                                        us2_t[:, m, :],
                                        usum_s[:, KAM, m, :, :],
                                        u2cm[:, :, :],
                                        start=(m == 0), stop=False,
                                        skip_group_check=True, perf_mode=DR,
                                    )
                            ps2t = s2hm[-1]
                            for gg in range(G):
                                ggl, ggh = gg * GB, (gg + 1) * GB
                                for m in range(C2):
                                    nc.tensor.matmul(
                                        us2_t[:, m, ggl:ggh],
                                        usum_s[:, p2x, m, :, :],
                                        ps2t[:, :, ggl:ggh],
                                        start=(tprev == 0 and gg == 0
                                               and m == 0),
                                        stop=(p2x == KAM - 1 and gg == G - 1
                                              and m == C2 - 1),
                                        skip_group_check=True, perf_mode=DR,
                                    )
                            if p1x == KAM - 1:
                                unew_g1_due[0] = True
                            if p2x == KAM - 1:
                                unew2_due[0] = True
                            pend_us[0] = None

                        # --- deferred merged L2 compare/reset for t-1 ---
                        if pend_s2[0] is not None:
                            pt, pj2p, p2p = pend_s2[0]
                            _L(f"s2n[{pt}]")
                            s2n = sp2.tile([128, C2, BC], f8t, tag="s2m",
                                           name="s2n")
                            nc.vector.scalar_tensor_tensor(
                                s2n[:], u2cm[:],
                                float(LAM2 * cb2 * rho2 ** (pj2p + 1)),
                                p2p[:], A.mult, A.is_lt,
                            )
                            _L(f"r2n[{pt}]")
                            r2n = rp.tile([128, C2, BC], f8t, tag="r2m",
                                          name="r2n")
                            nc.scalar.activation(
                                r2n[:], p2p[:], RELU,
                                scale=float(-1.0 / LAM2),
                            )
                            r2cm = r2n
                            s2hm.append(s2n)
                            if len(s2hm) > KAM + 1:
                                s2hm.pop(0)
                            pend_z[0] = (gi_tiles[pt // TP][:, pt % TP, :], s2n)
                            pend_us[0] = pend_us_next[0]
                            pend_s2[0] = None

                        # --- deferred z accumulation ---
                        if pend_z[0] is not None:
                            gsl, s2t = pend_z[0]
                            _L(f"z[{t-1}]")
                            for gg in range(G):
                                ggl, ggh = gg * GB, (gg + 1) * GB
                                for k in range(C2):
                                    nc.tensor.matmul(
                                        z_ps[:, k * BC + ggl : k * BC + ggh],
                                        gsl, s2t[:, k, ggl:ggh],
                                        start=zstart[0], stop=False,
                                        skip_group_check=True,
                                    )
                                    zstart[0] = False
                            pend_z[0] = None

                    # ----- P1 (scaled by LAM1) -----
                    _L(f"x1[{t}g{g}]")
                    p1 = ps1.tile([128, 2, 2, GB], fp32, tag=f"p1g{g}")
                    for m in range(C1):
                        nc.tensor.matmul(
                            p1[:, m // 2, m % 2, :], w1_s[:, :, m, :], xsl,
                            start=(m == 0), stop=False, perf_mode=DR,
                        )
                    _L(f"wrec[{t}g{g}]")
                    s1p = s1h[g][-1]
                    for m in range(C1):
                        for jj in range(2):
                            nc.tensor.matmul(
                                p1[:, m // 2, m % 2, :],
                                wr_s[:, jj, :, m, :], s1p[:, jj, :, :],
                                start=False, stop=False, perf_mode=DR,
                            )
                    _L(f"hist1[{t}g{g}]")
                    for i in range(2, min(j1g + 1, t) + 1):
                        sh = s1h[g][-i]
                        for m in range(C1):
                            nc.tensor.matmul(
                                p1[:, m // 2, m % 2, :],
                                hist_s[:, i - 1, m % 2, :, :],
                                sh[:, m // 2, :, :],
                                start=False, stop=False, perf_mode=DR,
                            )
                    _L(f"ai1[{t}g{g}]")
                    for m in range(C1):
                        nc.tensor.matmul(
                            p1[:, m // 2, m % 2, :],
                            nai_s[:, m % 2, :, :], r1c[g][:, m // 2, :, :],
                            start=False, stop=(m == C1 - 1), perf_mode=DR,
                        )

                    # ----- L1 compare (DVE) + reset (ACT) -----
                    _L(f"s1n[{t}g{g}]")
                    s1n = sp1.tile([128, 2, 2, GB], f8t, tag=f"s1g{g}")
                    nc.vector.scalar_tensor_tensor(
                        s1n[:], u1c[g][:], float(LAM1 * cb1 * rho1 ** (j1g + 1)),
                        p1[:], A.mult, A.is_lt,
                    )
                    _L(f"r1n[{t}g{g}]")
                    r1n = rp.tile([128, 2, 2, GB], f8t, tag=f"r1g{g}")
                    nc.scalar.activation(
                        r1n[:], p1[:], RELU, scale=float(-1.0 / LAM1)
                    )
                    r1c[g] = r1n
                    s1h[g].append(s1n)
                    if len(s1h[g]) > KAM + 1:
                        s1h[g].pop(0)


                    # ----- P2 (scaled by LAM2, shared bank, merged) -----
                    _L(f"a2r[{t}g{g}]")
                    if g == 0:
                        p2m = ps2.tile([128, C2, BC], fp32, tag="p2m")
                    p2v = p2m[:, :, gl:gh]
                    for m in range(C2):
                        nc.tensor.matmul(
                            p2v[:, m, :], a2r_s[:], ones_s[:, gl:gh],
                            start=(g == 0 and m == 0), stop=False,
                            skip_group_check=True,
                        )
                    _L(f"w2[{t}g{g}]")
                    for m in range(C2):
                        for jj in range(2):
                            nc.tensor.matmul(
                                p2v[:, m, :], w2_s[:, jj, :, m, :],
                                s1n[:, jj, :, :],
                                start=False, stop=False, perf_mode=DR,
                                skip_group_check=True,
                            )
                    _L(f"hist2[{t}g{g}]")
                    for i in range(2, min(j2g + 1, t) + 1):
                        sh = s2hm[-i]
                        for m in range(C2):
                            nc.tensor.matmul(
                                p2v[:, m, :], hist_s[:, i - 1, m, :, :],
                                sh[:, :, gl:gh],
                                start=False, stop=False, perf_mode=DR,
                                skip_group_check=True,
                            )
                    _L(f"ai2[{t}g{g}]")
                    s2p = s2hm[-1]
                    for m in range(C2):
                        nc.tensor.matmul(
                            p2v[:, m, :], nai_s[:, m, :, :],
                            r2cm[:, :, gl:gh],
                            start=False, stop=False, perf_mode=DR,
                            skip_group_check=True,
                        )
                        nc.tensor.matmul(
                            p2v[:, m, :], nai_s[:, m, :, :],
                            s2p[:, :, gl:gh],
                            start=False,
                            stop=(g == G - 1 and m == C2 - 1),
                            perf_mode=DR, skip_group_check=True,
                        )

                    if g == G - 1:
                        pend_s2[0] = (t, j2g, p2m)
                        pend_us_next[0] = (j1g, j2g, t)
                        if unew_g1_due[0]:
                            for gg in range(G):
                                ggl, ggh = gg * GB, (gg + 1) * GB
                                _L(f"unew[{t}g{gg}]")
                                u1n = up.tile([128, 2, 2, GB], f8t,
                                              tag=f"u1g{gg}",
                                              name=f"u1n{gg}")
                                nc.scalar.activation(
                                    u1n[:], us1_t[:, :, :, ggl:ggh], IDENT
                                )
                                u1c[gg] = u1n
                            unew_g1_due[0] = False
                        if unew2_due[0]:
                            _L(f"unew2[{t}]")
                            u2n = up.tile([128, C2, BC], f8t, tag="u2m")
                            nc.vector.tensor_scalar(
                                u2n[:], us2_t[:], 1.0, None, A.mult
                            )
                            u2cm = u2n
                            unew2_due[0] = False

            # ---- epilogue: flush deferred work, readout ----
            pt, pj2p, p2p = pend_s2[0]
            _L(f"s2n[{pt}]")
            s2fin = sp2.tile([128, C2, BC], f8t, tag="s2m", name="s2fin")
            nc.vector.scalar_tensor_tensor(
                s2fin[:], u2cm[:], float(LAM2 * cb2 * rho2 ** (pj2p + 1)),
                p2p[:], A.mult, A.is_lt,
            )
            _L("zfin")
            if pend_z[0] is not None:
                gsl, s2t = pend_z[0]
                for gg in range(G):
                    ggl, ggh = gg * GB, (gg + 1) * GB
                    for k in range(C2):
                        nc.tensor.matmul(
                            z_ps[:, k * BC + ggl : k * BC + ggh],
                            gsl, s2t[:, k, ggl:ggh],
                            start=False, stop=False,
                            skip_group_check=True,
                        )
            gslf = gi_tiles[(T - 1) // TP][:, (T - 1) % TP, :]
            for gg in range(G):
                ggl, ggh = gg * GB, (gg + 1) * GB
                for k in range(C2):
                    nc.tensor.matmul(
                        z_ps[:, k * BC + ggl : k * BC + ggh],
                        gslf, s2fin[:, k, ggl:ggh],
                        start=False,
                        stop=(gg == G - 1 and k == C2 - 1),
                        skip_group_check=True,
                    )

            _L("final")
            zb = tmp.tile([128, C2 * BC], bft, tag="zb")
            nc.scalar.activation(zb[:], z_ps[:], IDENT, scale=float(1.0 / LAMG))
            yo_t = ps2.tile([128, C2, BC], fp32, tag="p2m", name="yo_t")
            yo = yo_t[:N_OUT, 0, :]
            for k in range(C2):
                nc.tensor.matmul(
                    yo, w3_s[:, k, :], zb[:, k * BC : (k + 1) * BC],
                    start=(k == 0), stop=(k == C2 - 1),
                    skip_group_check=True,
                )
            outf = tmp.tile([N_OUT, BC], fp32, tag="outf")
            nc.vector.tensor_scalar(outf[:], yo, 1.0, None, A.mult)
            nc.sync.dma_start(out_d[:], outf[:])

    nc.compile()
    return nc


def _pack_rows61(a):
    """[122, ...] -> [64, 2, ...]: row r<61 at [r,0], 61<=r<122 at [r-61,1];
    partitions 61..63 zero."""
    out = np.zeros((64, 2) + a.shape[1:], a.dtype)
    out[:61, 0] = a[:61]
    out[:61, 1] = a[61:122]
    return out


def _prep_inputs(x, W1, Wrec, W2, W3, alpha1, rho1, beta_a1, alpha2, rho2, beta_a2, beta_out):
    a1 = float(np.asarray(alpha1).reshape(-1)[0])
    a2 = float(np.asarray(alpha2).reshape(-1)[0])
    bo = float(np.asarray(beta_out).reshape(-1)[0])
    rh1 = float(np.asarray(rho1).reshape(-1)[0])
    cb1 = float(np.asarray(beta_a1).reshape(-1)[0]) * (1.0 - rh1)

    w1s = ((1.0 - np.asarray(alpha1, np.float32)[:, None]) * np.asarray(W1, np.float32)).T
    wrs = ((1.0 - np.asarray(alpha1, np.float32)[:, None]) * np.asarray(Wrec, np.float32)).T
    w2s = ((1.0 - np.asarray(alpha2, np.float32)[:, None]) * np.asarray(W2, np.float32)).T
    w3s = np.asarray(W3, np.float32).T  # unscaled: (1-beta_out) folds into g_t

    wrs = (wrs - a1 * np.eye(H1, dtype=np.float32)) * LAM1
    w1aug = np.concatenate(
        [w1s, np.full((1, H1), a1 - 1.0, np.float32)], axis=0
    ) * LAM1  # [121, 512]
    w1aug = np.concatenate([w1aug, np.zeros((1, H1), np.float32)], axis=0)

    w1_a = _pack_rows61(w1aug.reshape(122, C1, 128)).astype(f8)
    wr_a = np.ascontiguousarray(
        wrs.reshape(2, 2, 128, C1, 128).transpose(2, 0, 1, 3, 4)
    ).astype(f8)
    w2_a = np.ascontiguousarray(
        (w2s * LAM2).reshape(2, 2, 128, C2, 128).transpose(2, 0, 1, 3, 4)
    ).astype(f8)
    w3_a = np.ascontiguousarray(
        w3s.reshape(C2, 128, N_OUT).transpose(1, 0, 2)
    ).astype(bf16)

    eye = np.eye(128, dtype=np.float32)

    def drpack(c):
        o = np.zeros((128, 2, 2, 128), np.float32)
        o[:, 0, 0] = c * eye
        o[:, 1, 1] = c * eye
        return o

    naidr = drpack(-a1 * LAM1).astype(f8)
    histdr = np.stack(
        [drpack(-LAM1 * cb1 * rh1 ** i) for i in range(KAM)], axis=1
    ).astype(f8)
    usumdr = np.stack(
        [drpack(rh1 ** (KAM - 1 - jj)) for jj in range(KAM)]
        + [drpack(rh1 ** KAM)], axis=1
    ).astype(f8)
    a2row = np.full((1, 128), (a2 - 1.0) * LAM2, np.float32).astype(f8)

    gvals = (1.0 - bo ** (T - np.arange(T, dtype=np.float64))) / T * LAMG
    gidt = (
        gvals[None, :, None].astype(np.float32) * eye[:, None, :]
    ).astype(f8)

    shared = dict(
        w1s=w1_a, wrecs=wr_a, w2s=w2_a, w3s=w3_a,
        naidr=naidr, histdr=histdr, usumdr=usumdr,
        a2row=a2row, gidt=gidt,
    )
    in_maps = []
    for c in range(N_CORES):
        xc = np.asarray(x[c * BC : (c + 1) * BC], np.float32)  # [BC, T, N_IN]
        xfm = xc.transpose(2, 1, 0)  # [N_IN, T, BC]
        xaug = np.concatenate(
            [xfm, np.ones((1, T, BC), np.float32),
             np.zeros((1, T, BC), np.float32)], axis=0
        )  # [122, T, BC]
        xp = _pack_rows61(xaug)
        in_maps.append(dict(x=np.ascontiguousarray(xp).astype(f8), **shared))
    return in_maps


def kernel(
    x, W1, Wrec, W2, W3,
    alpha1, rho1, beta_a1, alpha2, rho2, beta_a2, beta_out,
    _trace=False,
):
    from concourse.bass_utils import run_bass_kernel_spmd

    key = "nc"
    if key not in _CACHE:
        _CACHE[key] = _build(
            float(np.asarray(alpha1).reshape(-1)[0]),
            float(np.asarray(rho1).reshape(-1)[0]),
            float(np.asarray(beta_a1).reshape(-1)[0]),
            float(np.asarray(alpha2).reshape(-1)[0]),
            float(np.asarray(rho2).reshape(-1)[0]),
            float(np.asarray(beta_a2).reshape(-1)[0]),
            float(np.asarray(beta_out).reshape(-1)[0]),
        )
    nc = _CACHE[key]

    in_maps = _prep_inputs(
        x, W1, Wrec, W2, W3, alpha1, rho1, beta_a1, alpha2, rho2, beta_a2, beta_out
    )
    res = run_bass_kernel_spmd(nc, in_maps, list(range(N_CORES)), trace=_trace)

    out = np.empty((B, N_OUT), np.float32)
    for c in range(N_CORES):
        out[c * BC : (c + 1) * BC] = np.asarray(res.results[c]["out"]).T
    if _trace:
        return out, res
    return out
